# revision 9
# baseline (speedup 1.0000x reference)
"""Trainium2 kernel for the conditional optimal diffusion score
(per-class masked-softmax RBF regression over the dataset).

Math (see reference): for query u, dataset x (N,D), labels y (N,):
    inner_n = -(0.5/sigma2) * ||u - s*x_n||^2,  s = sqrt(alpha_bar[t])
    w = per-class softmax of inner over {n : y_n == c}        (K,N)
    combo_c = sum_n w_nc x_n                                   (K,D)
    out = -(1/sigma2) * (u - s*combo)                          (K,D)

Strategy: shard x/y row-wise over 8 NeuronCores.  Each core makes ONE
streaming pass over its x shard (the kernel is HBM-bandwidth bound):
  per 128-row tile: r = rowsum(x^2) on ScalarE (Square+accum),
  ux = rowsum(x*u) on VectorE (tensor_tensor_reduce), logits
  pre = c1*ux + c2*r, unnormalised weights e = exp(pre - M) with a
  per-core reference point M (max of tile-0 logits; exact softmax is
  restored at merge time because M cancels in V/S), one-hot masked
  weights W = (iota==y)*e, and PE matmuls accumulate V = W^T x (K,D)
  and S = W^T 1 (K,1) in PSUM across all tiles.
Host merges the 8 cores' (V_i, S_i, M_i) flash-attention style:
  combo = sum_i exp(M_i - M*) V_i / sum_i exp(M_i - M*) S_i.

The constant -(0.5/sigma2)*||u||^2 term of the logits is common to every
sample and every core, so it cancels in the softmax and is never computed.

Padding: shards are padded from 6250 to 6272 rows (49*128) with
x_pad = 1e15 (drives pre -> -inf -> e = 0) and y_pad = 100 (matches no
class, so W = 0 even when c1 = c2 = 0).
"""

import numpy as np

N, CH, HH, WW = 50000, 3, 32, 32
D = CH * HH * WW        # 3072
K = 10
NCORES = 8
NSHARD = N // NCORES    # 6250
P = 128
NT = 49                 # tiles per core
NPAD = NT * P           # 6272
FREE = 512              # matmul moving-operand slice (fp32 max)
NSLICE = D // FREE      # 6
PAD_X = 1.0e15
PAD_Y = 100.0
USE_F32R = False

_NC_CACHE = {}
LAST_RESULTS = None


def _build_nc(c1: float, c2: float, nt: int = NT):
    from contextlib import ExitStack

    import concourse.bacc as bacc
    import concourse.bass as bass
    import concourse.bass_isa as bass_isa
    import concourse.tile as tile
    from concourse import mybir

    f32 = mybir.dt.float32
    f32r = mybir.dt.float32r
    Alu = mybir.AluOpType
    Act = mybir.ActivationFunctionType

    nc = bacc.Bacc("TRN2", name="knn_softmax_score")

    npad = nt * P
    x_d = nc.dram_tensor("xs", [npad, D], f32, kind="ExternalInput")
    y_d = nc.dram_tensor("ys", [P, nt], f32, kind="ExternalInput")
    u_d = nc.dram_tensor("ub", [D], f32, kind="ExternalInput")
    i_d = nc.dram_tensor("iota10", [K], f32, kind="ExternalInput")
    v_d = nc.dram_tensor("v_out", [K, D], f32, kind="ExternalOutput")
    s_d = nc.dram_tensor("s_out", [K, 1], f32, kind="ExternalOutput")
    g_d = nc.dram_tensor("g_out", [1, 1], f32, kind="ExternalOutput")
    m_dram = nc.dram_tensor("m_scratch", [P], f32)

    with ExitStack() as ctx:
        tc = ctx.enter_context(tile.TileContext(nc))
        singles = ctx.enter_context(tc.tile_pool(name="singles", bufs=1))
        xpool = ctx.enter_context(tc.tile_pool(name="xpool", bufs=6))
        wpool = ctx.enter_context(tc.tile_pool(name="wpool", bufs=4))
        qpool = ctx.enter_context(tc.tile_pool(name="qpool", bufs=4))
        pspool = ctx.enter_context(tc.tile_pool(name="ps", bufs=1, space="PSUM"))

        # constants / broadcasts
        ub = singles.tile([P, D], f32, tag="ub")
        nc.gpsimd.dma_start(
            out=ub,
            in_=bass.AP(tensor=u_d[:].tensor, offset=0, ap=[[0, P], [1, D]]),
        )
        iota_row = singles.tile([P, K], f32, tag="iota")
        nc.gpsimd.dma_start(
            out=iota_row,
            in_=bass.AP(tensor=i_d[:].tensor, offset=0, ap=[[0, P], [1, K]]),
        )
        ysb = singles.tile([P, nt], f32, tag="ysb")
        nc.sync.dma_start(out=ysb, in_=y_d[:, :])
        ones_col = singles.tile([P, 1], f32, tag="ones")
        nc.vector.memset(ones_col, 1.0)

        dve_scr = singles.tile([P, D], f32, tag="dve_scr")
        act_scr = singles.tile([P, D], f32, tag="act_scr")
        r_all = singles.tile([P, nt], f32, tag="r_all")
        ux_all = singles.tile([P, nt], f32, tag="ux_all")
        pre_all = singles.tile([P, nt], f32, tag="pre_all")
        e_all = singles.tile([P, nt], f32, tag="e_all")
        m_col = singles.tile([P, 1], f32, tag="m_col")
        negm = singles.tile([P, 1], f32, tag="negm")
        vsb = singles.tile([K, D], f32, tag="vsb")
        ssb = singles.tile([K, 1], f32, tag="ssb")

        psV = [
            pspool.tile([K, FREE], f32, tag=f"v{j}", name=f"psV{j}")
            for j in range(NSLICE)
        ]
        psS = pspool.tile([K, 1], f32, tag="s")

        for t in range(nt):
            xt = xpool.tile([P, D], f32, tag="xt")
            nc.sync.dma_start(out=xt, in_=x_d[t * P : (t + 1) * P, :])

            rcol = r_all[:, t : t + 1]
            nc.scalar.activation(
                out=act_scr, in_=xt, func=Act.Square, accum_out=rcol
            )
            uxcol = ux_all[:, t : t + 1]
            nc.vector.scalar_tensor_tensor(
                out=dve_scr,
                in0=xt,
                scalar=1.0,
                op0=Alu.mult,
                in1=ub,
                op1=Alu.mult,
                accum_out=uxcol,
            )
            qcol = qpool.tile([P, 1], f32, tag="q")
            nc.vector.tensor_scalar(qcol, uxcol, c1, None, Alu.mult)
            pcol = pre_all[:, t : t + 1]
            nc.vector.tensor_scalar(pcol, rcol, c2, qcol[:, :], Alu.mult, Alu.add)
            if t == 0:
                # max over the 128 partitions of pcol: bounce through DRAM to
                # re-read the column as one 128-long row, reduce on DVE, then
                # broadcast the scalar back to all partitions.
                nc.sync.dma_start(out=m_dram[:], in_=pcol)
                m_row = singles.tile([1, P], f32, tag="m_row")
                nc.sync.dma_start(
                    out=m_row,
                    in_=bass.AP(tensor=m_dram[:].tensor, offset=0, ap=[[0, 1], [1, P]]),
                )
                m_scalar = singles.tile([1, 1], f32, tag="m_scalar")
                nc.vector.tensor_reduce(
                    m_scalar, m_row, axis=mybir.AxisListType.X, op=Alu.max
                )
                nc.gpsimd.partition_broadcast(m_col, m_scalar, channels=P)
                nc.vector.tensor_scalar(negm, m_col, -1.0, None, Alu.mult)
            ecol = e_all[:, t : t + 1]
            nc.scalar.activation(
                out=ecol, in_=pcol, func=Act.Exp, bias=negm[:, :], scale=1.0
            )
            wt = wpool.tile([P, K], f32, tag="wt")
            nc.vector.tensor_scalar(
                wt, iota_row, ysb[:, t : t + 1], ecol, Alu.is_equal, Alu.mult
            )

            lhs = wt[:, :].bitcast(f32r) if USE_F32R else wt[:, :]
            first, last = (t == 0), (t == nt - 1)
            for j in range(NSLICE):
                rhs = xt[:, j * FREE : (j + 1) * FREE]
                if USE_F32R:
                    rhs = rhs.bitcast(f32r)
                nc.tensor.matmul(psV[j], lhs, rhs, start=first, stop=last)
            rhs1 = ones_col[:, :].bitcast(f32r) if USE_F32R else ones_col[:, :]
            nc.tensor.matmul(psS, lhs, rhs1, start=first, stop=last)

        for j in range(NSLICE):
            nc.scalar.copy(out=vsb[:, j * FREE : (j + 1) * FREE], in_=psV[j][:, :])
        nc.vector.tensor_copy(ssb, psS[:, :])
        nc.sync.dma_start(out=v_d[:, :], in_=vsb)
        nc.sync.dma_start(out=s_d[:, :], in_=ssb)
        nc.sync.dma_start(out=g_d[:, :], in_=m_col[0:1, :])

    nc.finalize()
    return nc


def kernel(u, x_data, y, alpha_bar, t):
    from concourse.bass_utils import run_bass_kernel_spmd

    u = np.asarray(u, dtype=np.float32)
    x_data = np.asarray(x_data, dtype=np.float32)
    y = np.asarray(y)
    alpha_bar = np.asarray(alpha_bar, dtype=np.float32)
    ti = int(np.asarray(t))

    a_bar = float(alpha_bar[ti])
    s = float(np.sqrt(a_bar))
    sigma2 = 1.0 - a_bar
    c1 = s / sigma2
    c2 = -0.5 * s * s / sigma2

    key = (np.float32(c1).item(), np.float32(c2).item())
    if key not in _NC_CACHE:
        _NC_CACHE.clear()
        _NC_CACHE[key] = _build_nc(c1, c2)
    nc = _NC_CACHE[key]

    x_flat = x_data.reshape(N, D)
    u_flat = np.ascontiguousarray(u.reshape(D))
    iota10 = np.arange(K, dtype=np.float32)

    in_maps = []
    for i in range(NCORES):
        xs = np.full((NPAD, D), PAD_X, dtype=np.float32)
        xs[:NSHARD] = x_flat[i * NSHARD : (i + 1) * NSHARD]
        ys = np.full((NPAD,), PAD_Y, dtype=np.float32)
        ys[:NSHARD] = y[i * NSHARD : (i + 1) * NSHARD].astype(np.float32)
        ys = np.ascontiguousarray(ys.reshape(NT, P).T)  # [P, NT]
        in_maps.append({"xs": xs, "ys": ys, "ub": u_flat, "iota10": iota10})

    import os

    trace = os.environ.get("KNN_TRACE", "0") == "1"
    res = run_bass_kernel_spmd(
        nc, in_maps, core_ids=list(range(NCORES)), trace=trace
    )
    global LAST_RESULTS
    LAST_RESULTS = res

    # flash-attention style merge of the per-core softmax statistics
    Vs = np.stack([r["v_out"] for r in res.results]).astype(np.float64)
    Ss = np.stack([r["s_out"] for r in res.results]).astype(np.float64)
    Ms = np.array([float(r["g_out"][0, 0]) for r in res.results], dtype=np.float64)
    f = np.exp(Ms - Ms.max())
    V = np.einsum("i,ikd->kd", f, Vs)
    S = np.einsum("i,iko->ko", f, Ss)
    combo = V / S
    result = -(1.0 / sigma2) * (u_flat[None, :] - s * combo)
    return result.astype(np.float32).reshape(K, 1, CH, HH, WW)


# revision 13
# speedup vs baseline: 1.1441x; 1.1441x over previous
"""Trainium2 kernel for the conditional optimal diffusion score
(per-class masked-softmax RBF regression over the dataset).

Math (see reference): for query u, dataset x (N,D), labels y (N,):
    inner_n = -(0.5/sigma2) * ||u - s*x_n||^2,  s = sqrt(alpha_bar[t])
    w = per-class softmax of inner over {n : y_n == c}        (K,N)
    combo_c = sum_n w_nc x_n                                   (K,D)
    out = -(1/sigma2) * (u - s*combo)                          (K,D)

Strategy: shard x/y row-wise over 8 NeuronCores.  Each core makes ONE
streaming pass over its x shard (the kernel is HBM-bandwidth bound):
  per 128-row tile: r = rowsum(x^2) on ScalarE (Square+accum),
  ux = rowsum(x*u) on VectorE (tensor_tensor_reduce), logits
  pre = c1*ux + c2*r, unnormalised weights e = exp(pre - M) with a
  per-core reference point M (max of tile-0 logits; exact softmax is
  restored at merge time because M cancels in V/S), one-hot masked
  weights W = (iota==y)*e, and PE matmuls accumulate V = W^T x (K,D)
  and S = W^T 1 (K,1) in PSUM across all tiles.
Host merges the 8 cores' (V_i, S_i, M_i) flash-attention style:
  combo = sum_i exp(M_i - M*) V_i / sum_i exp(M_i - M*) S_i.

The constant -(0.5/sigma2)*||u||^2 term of the logits is common to every
sample and every core, so it cancels in the softmax and is never computed.

Padding: shards are padded from 6250 to 6272 rows (49*128) with
x_pad = 1e15 (drives pre -> -inf -> e = 0) and y_pad = 100 (matches no
class, so W = 0 even when c1 = c2 = 0).
"""

import numpy as np

N, CH, HH, WW = 50000, 3, 32, 32
D = CH * HH * WW        # 3072
K = 10
NCORES = 8
NSHARD = N // NCORES    # 6250
P = 128
NT = 49                 # tiles per core
NPAD = NT * P           # 6272
FREE = 512              # matmul moving-operand slice (fp32 max)
NSLICE = D // FREE      # 6
PAD_X = 1.0e15
PAD_Y = 100.0
USE_F32R = True

_NC_CACHE = {}
LAST_RESULTS = None


def _build_nc(c1: float, c2: float, nt: int = NT):
    from contextlib import ExitStack

    import concourse.bacc as bacc
    import concourse.bass as bass
    import concourse.bass_isa as bass_isa
    import concourse.tile as tile
    from concourse import mybir

    f32 = mybir.dt.float32
    f32r = mybir.dt.float32r
    Alu = mybir.AluOpType
    Act = mybir.ActivationFunctionType

    nc = bacc.Bacc("TRN2", name="knn_softmax_score")

    npad = nt * P
    xdt = f32r if USE_F32R else f32
    x_d = nc.dram_tensor("xs", [npad, D], xdt, kind="ExternalInput")
    y_d = nc.dram_tensor("ys", [P, nt], f32, kind="ExternalInput")
    u_d = nc.dram_tensor("ub", [D], f32, kind="ExternalInput")
    i_d = nc.dram_tensor("iota10", [K], f32, kind="ExternalInput")

    v_d = nc.dram_tensor("v_out", [K, D], f32, kind="ExternalOutput")
    s_d = nc.dram_tensor("s_out", [K, 1], f32, kind="ExternalOutput")
    g_d = nc.dram_tensor("g_out", [1, 1], f32, kind="ExternalOutput")
    m_dram = nc.dram_tensor("m_scratch", [P], f32)

    with ExitStack() as ctx:
        tc = ctx.enter_context(tile.TileContext(nc))
        singles = ctx.enter_context(tc.tile_pool(name="singles", bufs=1))
        xpool = ctx.enter_context(tc.tile_pool(name="xpool", bufs=6))
        wpool = ctx.enter_context(tc.tile_pool(name="wpool", bufs=4))
        qpool = ctx.enter_context(tc.tile_pool(name="qpool", bufs=4))
        pspool = ctx.enter_context(tc.tile_pool(name="ps", bufs=1, space="PSUM"))

        # constants / broadcasts
        ub = singles.tile([P, D], f32, tag="ub")
        nc.gpsimd.dma_start(
            out=ub,
            in_=bass.AP(tensor=u_d[:].tensor, offset=0, ap=[[0, P], [1, D]]),
        )
        iota_row = singles.tile([P, K], f32, tag="iota")
        nc.gpsimd.dma_start(
            out=iota_row,
            in_=bass.AP(tensor=i_d[:].tensor, offset=0, ap=[[0, P], [1, K]]),
        )
        ysb = singles.tile([P, nt], f32, tag="ysb")
        nc.sync.dma_start(out=ysb, in_=y_d[:, :])
        ones_col = singles.tile([P, 1], f32, tag="ones")
        nc.vector.memset(ones_col, 1.0)

        dve_scr = singles.tile([P, D], f32, tag="dve_scr")
        act_scr = singles.tile([P, D], f32, tag="act_scr")
        r_all = singles.tile([P, nt], f32, tag="r_all")
        ux_all = singles.tile([P, nt], f32, tag="ux_all")
        pre_all = singles.tile([P, nt], f32, tag="pre_all")
        e_all = singles.tile([P, nt], f32, tag="e_all")
        m_col = singles.tile([P, 1], f32, tag="m_col")
        negm = singles.tile([P, 1], f32, tag="negm")
        vsb = singles.tile([K, D], f32, tag="vsb")
        ssb = singles.tile([K, 1], f32, tag="ssb")

        psV = [
            pspool.tile([K, FREE], f32, tag=f"v{j}", name=f"psV{j}")
            for j in range(NSLICE)
        ]
        psS = pspool.tile([K, 1], f32, tag="s")

        for t in range(nt):
            xt = xpool.tile([P, D], xdt, tag="xt")
            nc.sync.dma_start(out=xt, in_=x_d[t * P : (t + 1) * P, :])
            xt_f = xt[:, :].bitcast(f32)

            rcol = r_all[:, t : t + 1]
            nc.scalar.activation(
                out=act_scr, in_=xt_f, func=Act.Square, accum_out=rcol
            )
            uxcol = ux_all[:, t : t + 1]
            nc.vector.scalar_tensor_tensor(
                out=dve_scr,
                in0=xt_f,
                scalar=1.0,
                op0=Alu.mult,
                in1=ub,
                op1=Alu.mult,
                accum_out=uxcol,
            )
            qcol = qpool.tile([P, 1], f32, tag="q")
            nc.vector.tensor_scalar(qcol, uxcol, c1, None, Alu.mult)
            pcol = pre_all[:, t : t + 1]
            nc.vector.tensor_scalar(pcol, rcol, c2, qcol[:, :], Alu.mult, Alu.add)
            if t == 0:
                # max over the 128 partitions of pcol: bounce through DRAM to
                # re-read the column as one 128-long row, reduce on DVE, then
                # broadcast the scalar back to all partitions.
                nc.sync.dma_start(out=m_dram[:], in_=pcol)
                m_row = singles.tile([1, P], f32, tag="m_row")
                nc.sync.dma_start(
                    out=m_row,
                    in_=bass.AP(tensor=m_dram[:].tensor, offset=0, ap=[[0, 1], [1, P]]),
                )
                m_scalar = singles.tile([1, 1], f32, tag="m_scalar")
                nc.vector.tensor_reduce(
                    m_scalar, m_row, axis=mybir.AxisListType.X, op=Alu.max
                )
                nc.gpsimd.partition_broadcast(m_col, m_scalar, channels=P)
                nc.vector.tensor_scalar(negm, m_col, -1.0, None, Alu.mult)
            ecol = e_all[:, t : t + 1]
            nc.scalar.activation(
                out=ecol, in_=pcol, func=Act.Exp, bias=negm[:, :], scale=1.0
            )
            wt = wpool.tile([P, K], xdt, tag="wt")
            nc.vector.tensor_scalar(
                wt, iota_row, ysb[:, t : t + 1], ecol, Alu.is_equal, Alu.mult
            )

            lhs = wt[:, :]
            first, last = (t == 0), (t == nt - 1)
            for j in range(NSLICE):
                rhs = xt[:, j * FREE : (j + 1) * FREE]
                nc.tensor.matmul(psV[j], lhs, rhs, start=first, stop=last)
            nc.tensor.matmul(
                psS, wt[:, :].bitcast(f32), ones_col[:, :], start=first, stop=last
            )

        for j in range(NSLICE):
            nc.scalar.copy(out=vsb[:, j * FREE : (j + 1) * FREE], in_=psV[j][:, :])
        nc.vector.tensor_copy(ssb, psS[:, :])
        nc.sync.dma_start(out=v_d[:, :], in_=vsb)
        nc.sync.dma_start(out=s_d[:, :], in_=ssb)
        nc.sync.dma_start(out=g_d[:, :], in_=m_col[0:1, :])

    nc.finalize()
    return nc


def kernel(u, x_data, y, alpha_bar, t):
    from concourse.bass_utils import run_bass_kernel_spmd

    u = np.asarray(u, dtype=np.float32)
    x_data = np.asarray(x_data, dtype=np.float32)
    y = np.asarray(y)
    alpha_bar = np.asarray(alpha_bar, dtype=np.float32)
    ti = int(np.asarray(t))

    a_bar = float(alpha_bar[ti])
    s = float(np.sqrt(a_bar))
    sigma2 = 1.0 - a_bar
    c1 = s / sigma2
    c2 = -0.5 * s * s / sigma2

    key = (np.float32(c1).item(), np.float32(c2).item())
    if key not in _NC_CACHE:
        _NC_CACHE.clear()
        _NC_CACHE[key] = _build_nc(c1, c2)
    nc = _NC_CACHE[key]

    x_flat = x_data.reshape(N, D)
    u_flat = np.ascontiguousarray(u.reshape(D))
    iota10 = np.arange(K, dtype=np.float32)

    in_maps = []
    for i in range(NCORES):
        xs = np.full((NPAD, D), PAD_X, dtype=np.float32)
        xs[:NSHARD] = x_flat[i * NSHARD : (i + 1) * NSHARD]
        ys = np.full((NPAD,), PAD_Y, dtype=np.float32)
        ys[:NSHARD] = y[i * NSHARD : (i + 1) * NSHARD].astype(np.float32)
        ys = np.ascontiguousarray(ys.reshape(NT, P).T)  # [P, NT]
        in_maps.append(
            {
                "xs": xs,
                "ys": ys,
                "ub": u_flat,
                "iota10": iota10,
            }
        )

    import os

    trace = os.environ.get("KNN_TRACE", "0") == "1"
    res = run_bass_kernel_spmd(
        nc, in_maps, core_ids=list(range(NCORES)), trace=trace
    )
    global LAST_RESULTS
    LAST_RESULTS = res

    # flash-attention style merge of the per-core softmax statistics
    Vs = np.stack([r["v_out"] for r in res.results]).astype(np.float64)
    Ss = np.stack([r["s_out"] for r in res.results]).astype(np.float64)
    Ms = np.array([float(r["g_out"][0, 0]) for r in res.results], dtype=np.float64)
    f = np.exp(Ms - Ms.max())
    V = np.einsum("i,ikd->kd", f, Vs)
    S = np.einsum("i,iko->ko", f, Ss)
    combo = V / S
    result = -(1.0 / sigma2) * (u_flat[None, :] - s * combo)
    return result.astype(np.float32).reshape(K, 1, CH, HH, WW)


# revision 14
# speedup vs baseline: 1.1826x; 1.0336x over previous
"""Trainium2 kernel for the conditional optimal diffusion score
(per-class masked-softmax RBF regression over the dataset).

Math (see reference): for query u, dataset x (N,D), labels y (N,):
    inner_n = -(0.5/sigma2) * ||u - s*x_n||^2,  s = sqrt(alpha_bar[t])
    w = per-class softmax of inner over {n : y_n == c}        (K,N)
    combo_c = sum_n w_nc x_n                                   (K,D)
    out = -(1/sigma2) * (u - s*combo)                          (K,D)

Strategy: shard x/y row-wise over 8 NeuronCores.  Each core makes ONE
streaming pass over its x shard (the kernel is HBM-bandwidth bound):
  per 128-row tile: r = rowsum(x^2) on ScalarE (Square+accum),
  ux = rowsum(x*u) on VectorE (tensor_tensor_reduce), logits
  pre = c1*ux + c2*r, unnormalised weights e = exp(pre - M) with a
  per-core reference point M (max of tile-0 logits; exact softmax is
  restored at merge time because M cancels in V/S), one-hot masked
  weights W = (iota==y)*e, and PE matmuls accumulate V = W^T x (K,D)
  and S = W^T 1 (K,1) in PSUM across all tiles.
Host merges the 8 cores' (V_i, S_i, M_i) flash-attention style:
  combo = sum_i exp(M_i - M*) V_i / sum_i exp(M_i - M*) S_i.

The constant -(0.5/sigma2)*||u||^2 term of the logits is common to every
sample and every core, so it cancels in the softmax and is never computed.

Padding: shards are padded from 6250 to 6272 rows (49*128) with
x_pad = 1e15 (drives pre -> -inf -> e = 0) and y_pad = 100 (matches no
class, so W = 0 even when c1 = c2 = 0).
"""

import numpy as np

N, CH, HH, WW = 50000, 3, 32, 32
D = CH * HH * WW        # 3072
K = 10
NCORES = 8
NSHARD = N // NCORES    # 6250
P = 128
NT = 49                 # tiles per core
NPAD = NT * P           # 6272
FREE = 512              # matmul moving-operand slice (fp32 max)
NSLICE = D // FREE      # 6
PAD_X = 1.0e15
PAD_Y = 100.0
USE_F32R = True

_NC_CACHE = {}
LAST_RESULTS = None


def _build_nc(c1: float, c2: float, nt: int = NT):
    # fold: e = exp(c1*pre' - c1*m'), pre' = (c2/c1)*r + ux  (c1 >= 0 always;
    # for c1 == 0 both coefficients vanish and the softmax is uniform)
    rc = (c2 / c1) if c1 > 0.0 else 0.0
    esc = c1 if c1 > 0.0 else 1.0
    from contextlib import ExitStack

    import concourse.bacc as bacc
    import concourse.bass as bass
    import concourse.bass_isa as bass_isa
    import concourse.tile as tile
    from concourse import mybir

    f32 = mybir.dt.float32
    f32r = mybir.dt.float32r
    Alu = mybir.AluOpType
    Act = mybir.ActivationFunctionType

    nc = bacc.Bacc("TRN2", name="knn_softmax_score")

    npad = nt * P
    xdt = f32r if USE_F32R else f32
    x_d = nc.dram_tensor("xs", [npad, D], xdt, kind="ExternalInput")
    y_d = nc.dram_tensor("ys", [P, nt], f32, kind="ExternalInput")
    u_d = nc.dram_tensor("ub", [D], f32, kind="ExternalInput")
    i_d = nc.dram_tensor("iota10", [K], f32, kind="ExternalInput")

    v_d = nc.dram_tensor("v_out", [K, D], f32, kind="ExternalOutput")
    s_d = nc.dram_tensor("s_out", [K, 1], f32, kind="ExternalOutput")
    g_d = nc.dram_tensor("g_out", [1, 1], f32, kind="ExternalOutput")
    m_dram = nc.dram_tensor("m_scratch", [P], f32)

    with ExitStack() as ctx:
        tc = ctx.enter_context(tile.TileContext(nc))
        singles = ctx.enter_context(tc.tile_pool(name="singles", bufs=1))
        xpool = ctx.enter_context(tc.tile_pool(name="xpool", bufs=10))
        wpool = ctx.enter_context(tc.tile_pool(name="wpool", bufs=4))
        pspool = ctx.enter_context(tc.tile_pool(name="ps", bufs=1, space="PSUM"))

        # constants / broadcasts
        ub = singles.tile([P, D], f32, tag="ub")
        nc.gpsimd.dma_start(
            out=ub,
            in_=bass.AP(tensor=u_d[:].tensor, offset=0, ap=[[0, P], [1, D]]),
        )
        iota_row = singles.tile([P, K], f32, tag="iota")
        nc.gpsimd.dma_start(
            out=iota_row,
            in_=bass.AP(tensor=i_d[:].tensor, offset=0, ap=[[0, P], [1, K]]),
        )
        ysb = singles.tile([P, nt], f32, tag="ysb")
        nc.sync.dma_start(out=ysb, in_=y_d[:, :])
        ones_col = singles.tile([P, 1], f32, tag="ones")
        nc.vector.memset(ones_col, 1.0)

        dve_scr = singles.tile([P, D], f32, tag="dve_scr")
        act_scr = singles.tile([P, D], f32, tag="act_scr")
        r_all = singles.tile([P, nt], f32, tag="r_all")
        ux_all = singles.tile([P, nt], f32, tag="ux_all")
        pre_all = singles.tile([P, nt], f32, tag="pre_all")
        e_all = singles.tile([P, nt], f32, tag="e_all")
        m_col = singles.tile([P, 1], f32, tag="m_col")
        negm = singles.tile([P, 1], f32, tag="negm")
        vsb = singles.tile([K, D], f32, tag="vsb")
        ssb = singles.tile([K, 1], f32, tag="ssb")

        psV = [
            pspool.tile([K, FREE], f32, tag=f"v{j}", name=f"psV{j}")
            for j in range(NSLICE)
        ]
        psS = pspool.tile([K, 1], f32, tag="s")

        for t in range(nt):
            xt = xpool.tile([P, D], xdt, tag="xt")
            nc.sync.dma_start(out=xt, in_=x_d[t * P : (t + 1) * P, :])
            xt_f = xt[:, :].bitcast(f32)

            rcol = r_all[:, t : t + 1]
            nc.scalar.activation(
                out=act_scr, in_=xt_f, func=Act.Square, accum_out=rcol
            )
            uxcol = ux_all[:, t : t + 1]
            nc.vector.scalar_tensor_tensor(
                out=dve_scr,
                in0=xt_f,
                scalar=1.0,
                op0=Alu.mult,
                in1=ub,
                op1=Alu.mult,
                accum_out=uxcol,
            )
            pcol = pre_all[:, t : t + 1]
            nc.vector.tensor_scalar(pcol, rcol, rc, uxcol, Alu.mult, Alu.add)
            if t == 0:
                # max over the 128 partitions of pcol: bounce through DRAM to
                # re-read the column as one 128-long row, reduce on DVE, then
                # broadcast the scalar back to all partitions.
                nc.sync.dma_start(out=m_dram[:], in_=pcol)
                m_row = singles.tile([1, P], f32, tag="m_row")
                nc.sync.dma_start(
                    out=m_row,
                    in_=bass.AP(tensor=m_dram[:].tensor, offset=0, ap=[[0, 1], [1, P]]),
                )
                m_scalar = singles.tile([1, 1], f32, tag="m_scalar")
                nc.vector.tensor_reduce(
                    m_scalar, m_row, axis=mybir.AxisListType.X, op=Alu.max
                )
                nc.gpsimd.partition_broadcast(m_col, m_scalar, channels=P)
                nc.vector.tensor_scalar(negm, m_col, -esc, None, Alu.mult)
            ecol = e_all[:, t : t + 1]
            nc.scalar.activation(
                out=ecol, in_=pcol, func=Act.Exp, bias=negm[:, :], scale=esc
            )
            wt = wpool.tile([P, K], xdt, tag="wt")
            nc.vector.tensor_scalar(
                wt, iota_row, ysb[:, t : t + 1], ecol, Alu.is_equal, Alu.mult
            )

            lhs = wt[:, :]
            first, last = (t == 0), (t == nt - 1)
            for j in range(NSLICE):
                rhs = xt[:, j * FREE : (j + 1) * FREE]
                nc.tensor.matmul(psV[j], lhs, rhs, start=first, stop=last)
            nc.tensor.matmul(
                psS, wt[:, :].bitcast(f32), ones_col[:, :], start=first, stop=last
            )

        for j in range(NSLICE):
            dst = vsb[:, j * FREE : (j + 1) * FREE]
            if j % 2 == 0:
                nc.scalar.copy(out=dst, in_=psV[j][:, :])
            else:
                nc.vector.tensor_copy(dst, psV[j][:, :])
        nc.vector.tensor_copy(ssb, psS[:, :])
        nc.sync.dma_start(out=v_d[:, :], in_=vsb)
        nc.sync.dma_start(out=s_d[:, :], in_=ssb)
        nc.sync.dma_start(out=g_d[:, :], in_=m_col[0:1, :])

    nc.finalize()
    return nc


def kernel(u, x_data, y, alpha_bar, t):
    from concourse.bass_utils import run_bass_kernel_spmd

    u = np.asarray(u, dtype=np.float32)
    x_data = np.asarray(x_data, dtype=np.float32)
    y = np.asarray(y)
    alpha_bar = np.asarray(alpha_bar, dtype=np.float32)
    ti = int(np.asarray(t))

    a_bar = float(alpha_bar[ti])
    s = float(np.sqrt(a_bar))
    sigma2 = 1.0 - a_bar
    c1 = s / sigma2
    c2 = -0.5 * s * s / sigma2

    key = (np.float32(c1).item(), np.float32(c2).item())
    if key not in _NC_CACHE:
        _NC_CACHE.clear()
        _NC_CACHE[key] = _build_nc(c1, c2)
    nc = _NC_CACHE[key]

    x_flat = x_data.reshape(N, D)
    u_flat = np.ascontiguousarray(u.reshape(D))
    iota10 = np.arange(K, dtype=np.float32)

    in_maps = []
    for i in range(NCORES):
        xs = np.full((NPAD, D), PAD_X, dtype=np.float32)
        xs[:NSHARD] = x_flat[i * NSHARD : (i + 1) * NSHARD]
        ys = np.full((NPAD,), PAD_Y, dtype=np.float32)
        ys[:NSHARD] = y[i * NSHARD : (i + 1) * NSHARD].astype(np.float32)
        ys = np.ascontiguousarray(ys.reshape(NT, P).T)  # [P, NT]
        in_maps.append(
            {
                "xs": xs,
                "ys": ys,
                "ub": u_flat,
                "iota10": iota10,
            }
        )

    import os

    trace = os.environ.get("KNN_TRACE", "0") == "1"
    res = run_bass_kernel_spmd(
        nc, in_maps, core_ids=list(range(NCORES)), trace=trace
    )
    global LAST_RESULTS
    LAST_RESULTS = res

    # flash-attention style merge of the per-core softmax statistics
    Vs = np.stack([r["v_out"] for r in res.results]).astype(np.float64)
    Ss = np.stack([r["s_out"] for r in res.results]).astype(np.float64)
    Ms = np.array([float(r["g_out"][0, 0]) for r in res.results], dtype=np.float64)
    Ms = Ms * (c1 if c1 > 0.0 else 1.0)
    f = np.exp(Ms - Ms.max())
    V = np.einsum("i,ikd->kd", f, Vs)
    S = np.einsum("i,iko->ko", f, Ss)
    combo = V / S
    result = -(1.0 / sigma2) * (u_flat[None, :] - s * combo)
    return result.astype(np.float32).reshape(K, 1, CH, HH, WW)
